# revision 19
# baseline (speedup 1.0000x reference)
"""Trainium2 Bass kernel for BasicEdgeModel (edge-wise MLP with node gathers).

y[e] = relu(concat(x[src_e], x[tgt_e], edge_attr[e]) @ W1 + b1) @ W2 + b2

Strategy (8 NeuronCores, data-parallel over edges):
  - x.T (f32) lives in DRAM; node-bucket slices (<=32768 nodes) are staged
    into an SBUF table T [128, 32768] f32: rows 0-63 = src-bucket slice,
    rows 64-127 = tgt-bucket slice.
  - GPSIMD ap_gather does the per-edge gather entirely on-chip: each Q7
    core sweeps its own index stream (cores 0-3 src indices, cores 4-7 tgt
    indices), producing g[f, e] = merged-AB block [128 feats, E] directly —
    features on partitions, no transposes, ~1.9ns/edge.
  - Edges are sorted on host into 16 (src_bucket, tgt_bucket) groups on a
    fixed-capacity grid; groups snake so only one table half reloads.
  - Per 512-edge block: PSUM = W1AB.T @ g (float32r, full-rate) +
    W1C.T @ eaT (bf16); ACT fuses bias+relu into bf16 hT; W2.T @ hT gives
    [64, 512] blocks; pairs are stacked on partitions into a packed
    [128, E_GRID/2] f32 output. Host decodes + unpermutes.
"""

import numpy as np
import ml_dtypes

import concourse.bass as bass
import concourse.mybir as mybir
import concourse.tile as tile
from concourse import bacc
from concourse.bass_utils import run_bass_kernel_spmd

# problem geometry (fixed by the task)
N_NODES = 100000
NODE_DIM = 64
EDGE_DIM = 32
HIDDEN = 128
OUT_DIM = 64
N_EDGES = 1600000
N_CORES = 8
E_CORE = N_EDGES // N_CORES   # 200000

BUCKET = 32768                # nodes per table slice (int16 + ap_gather cap)
N_BUCKET = 4                  # ceil(100000 / 32768)
N_GROUP = N_BUCKET * N_BUCKET

BF16 = mybir.dt.bfloat16
F32 = mybir.dt.float32
F32R = mybir.dt.float32r
I16 = mybir.dt.int16
AF = mybir.ActivationFunctionType

TRACE = False
TRACE_TMPDIR = None
LAST_RESULT = None


def _bucket_width(b, n_nodes=N_NODES):
    return min(n_nodes - b * BUCKET, BUCKET)


def _group_caps(e_core=E_CORE, n_nodes=N_NODES):
    """Per-group slot capacities: mean + >8 sigma, rounded to 512."""
    p = np.array([_bucket_width(b, n_nodes) for b in range(N_BUCKET)]) / n_nodes
    caps = []
    for bs in range(N_BUCKET):
        for bt in range(N_BUCKET):
            mean = e_core * p[bs] * p[bt]
            sig = np.sqrt(max(mean, 1.0))
            # multiples of 1024 so every group spans an even number of
            # 512-blocks: output block-pairs then never cross groups
            cap = int(np.ceil((mean + 8 * sig + 256) / 1024) * 1024)
            caps.append(max(cap, 1024))
    return caps


def _segments(cap, seg_max):
    segs = []
    rem = cap
    while rem > 0:
        s = min(rem, seg_max)
        segs.append(s)
        rem -= s
    assert all(x % 128 == 0 for x in segs)
    return segs


def _snake_groups():
    """(bs, bt) order minimizing table-half reloads."""
    order = []
    for bs in range(N_BUCKET):
        bts = range(N_BUCKET) if bs % 2 == 0 else range(N_BUCKET - 1, -1, -1)
        for bt in bts:
            order.append((bs, bt))
    return order


GROUP_CAPS = _group_caps()
SEG_MAX = 4096
E_GRID = sum(GROUP_CAPS)


def build_nc(n_nodes, caps, seg_max):
    e_grid = sum(caps)
    assert e_grid % 1024 == 0
    bases = np.concatenate([[0], np.cumsum(caps)[:-1]]).astype(int)

    # segment table in snake order: (bs, bt, slot_base, seg_len, idx_col)
    segtab = []
    icols = 0
    for (bs, bt) in _snake_groups():
        g = bs * N_BUCKET + bt
        off = 0
        for L in _segments(caps[g], seg_max):
            segtab.append((bs, bt, int(bases[g]) + off, L, icols))
            icols += L // 16
            off += L

    nc = bacc.Bacc()
    xtf = nc.declare_dram_parameter("xtf", [NODE_DIM, n_nodes], F32, isOutput=False)
    idx = nc.declare_dram_parameter("idx", [128, icols], I16, isOutput=False)
    eat = nc.declare_dram_parameter("eat", [EDGE_DIM, e_grid], BF16, isOutput=False)
    w1ab = nc.declare_dram_parameter("w1ab", [2 * NODE_DIM, HIDDEN], BF16, isOutput=False)
    w1c = nc.declare_dram_parameter("w1c", [EDGE_DIM, HIDDEN], BF16, isOutput=False)
    w2 = nc.declare_dram_parameter("w2", [HIDDEN, OUT_DIM], BF16, isOutput=False)
    b1 = nc.declare_dram_parameter("b1", [HIDDEN, 1], F32, isOutput=False)
    b2 = nc.declare_dram_parameter("b2", [OUT_DIM, 1], F32, isOutput=False)
    out = nc.declare_dram_parameter("out", [128, e_grid // 2], F32, isOutput=True)

    with tile.TileContext(nc) as tc:
        with (
            tc.tile_pool(name="const", bufs=1) as cp,
            tc.tile_pool(name="tabp", bufs=1) as tabp,
            tc.tile_pool(name="idxp", bufs=2) as idxp,
            tc.tile_pool(name="gap", bufs=2) as gap,
            tc.tile_pool(name="eap", bufs=2) as eap,
            tc.tile_pool(name="htp", bufs=4) as htp,
            tc.tile_pool(name="osp", bufs=4) as osp,
            tc.tile_pool(name="hps", bufs=4, space="PSUM") as hps,
            tc.tile_pool(name="ops", bufs=4, space="PSUM") as ops,
        ):
            w1ab_t = cp.tile([2 * NODE_DIM, HIDDEN], BF16)
            nc.sync.dma_start(out=w1ab_t[:], in_=w1ab[:])
            w1c_t = cp.tile([EDGE_DIM, HIDDEN], BF16)
            nc.sync.dma_start(out=w1c_t[:], in_=w1c[:])
            w2_t = cp.tile([HIDDEN, OUT_DIM], BF16)
            nc.sync.dma_start(out=w2_t[:], in_=w2[:])
            b1_t = cp.tile([HIDDEN, 1], F32)
            nc.sync.dma_start(out=b1_t[:], in_=b1[:])
            b2_t = cp.tile([OUT_DIM, 1], F32)
            nc.sync.dma_start(out=b2_t[:], in_=b2[:])

            tabT = tabp.tile([128, BUCKET], F32)

            cur_bs, cur_bt = -1, -1
            for (bs, bt, slot_base, seg_len, icol) in segtab:
                if bs != cur_bs:
                    w = _bucket_width(bs, n_nodes)
                    nc.sync.dma_start(
                        out=tabT[0:NODE_DIM, :w],
                        in_=xtf[:, bs * BUCKET:bs * BUCKET + w],
                    )
                    cur_bs = bs
                if bt != cur_bt:
                    w = _bucket_width(bt, n_nodes)
                    nc.sync.dma_start(
                        out=tabT[NODE_DIM:2 * NODE_DIM, :w],
                        in_=xtf[:, bt * BUCKET:bt * BUCKET + w],
                    )
                    cur_bt = bt

                nw = seg_len // 16
                ix_t = idxp.tile([128, nw], I16, padded_shape=[128, seg_max // 16])
                nc.sync.dma_start(out=ix_t[:], in_=idx[:, icol:icol + nw])

                g_t = gap.tile([128, seg_len], F32,
                               padded_shape=[128, seg_max])
                nc.gpsimd.ap_gather(
                    g_t[:].rearrange("p (n d) -> p n d", d=1),
                    tabT[:].rearrange("p (n d) -> p n d", d=1), ix_t[:],
                    channels=128, num_elems=BUCKET, d=1, num_idxs=seg_len,
                )

                ea_t = eap.tile([EDGE_DIM, seg_len], BF16,
                                padded_shape=[EDGE_DIM, seg_max])
                nc.sync.dma_start(
                    out=ea_t[:], in_=eat[:, slot_base:slot_base + seg_len]
                )

                for b in range(seg_len // 512):
                    blk = slot_base // 512 + b
                    sl = slice(b * 512, (b + 1) * 512)
                    gbf = htp.tile([128, 512], BF16, tag="gbf")
                    nc.vector.tensor_copy(out=gbf[:], in_=g_t[:, sl])
                    hp = hps.tile([128, 512], F32, space="PSUM")
                    nc.tensor.matmul(
                        hp[:], lhsT=w1ab_t[:],
                        rhs=gbf[:],
                        start=True, stop=False,
                    )
                    nc.tensor.matmul(
                        hp[:], lhsT=w1c_t[:], rhs=ea_t[:, sl],
                        start=False, stop=True,
                    )
                    hT = htp.tile([128, 512], BF16)
                    nc.scalar.activation(
                        out=hT[:], in_=hp[:], func=AF.Relu,
                        bias=b1_t[:, :1], scale=1.0,
                    )
                    op = ops.tile([OUT_DIM, 512], F32, space="PSUM")
                    nc.tensor.matmul(
                        op[:], lhsT=w2_t[:], rhs=hT[:], start=True, stop=True,
                    )
                    j = blk % 2
                    if j == 0:
                        st = osp.tile([128, 512], F32)
                    nc.vector.tensor_tensor(
                        out=st[j * OUT_DIM:(j + 1) * OUT_DIM, :],
                        in0=op[:],
                        in1=b2_t[:, :1].to_broadcast([OUT_DIM, 512]),
                        op=mybir.AluOpType.add,
                    )
                    if j == 1:
                        col = (blk // 2) * 512
                        nc.sync.dma_start(out=out[:, col:col + 512], in_=st[:])

    nc.compile()
    return nc


def _wrap16(v):
    """[n] int -> [16, n/16] int16 (idx j at [j%16, j//16])."""
    n = v.shape[0]
    return v.reshape(n // 16, 16).T.astype(np.int16)


def _prep_core(src, tgt, ea, n_nodes, caps, seg_max):
    """Sort this core's edges into the grid; emit per-core-wrapped indices."""
    e_grid = sum(caps)
    n = src.shape[0]

    grp = (src >> 15) * N_BUCKET + (tgt >> 15)
    order = np.argsort(grp, kind="stable")
    counts = np.bincount(grp, minlength=N_GROUP)
    if np.any(counts > np.asarray(caps)):
        raise RuntimeError(f"group overflow: {counts} vs {caps}")

    bases = np.concatenate([[0], np.cumsum(caps)[:-1]]).astype(int)
    slot_of_sorted = np.empty(n, np.int64)
    start = 0
    for g in range(N_GROUP):
        c = counts[g]
        slot_of_sorted[start:start + c] = bases[g] + np.arange(c)
        start += c
    slot_of_edge = np.empty(n, np.int64)
    slot_of_edge[order] = slot_of_sorted

    srcs = np.zeros(e_grid, np.int32)
    tgts = np.zeros(e_grid, np.int32)
    for g in range(N_GROUP):
        srcs[bases[g]:bases[g] + caps[g]] = (g // N_BUCKET) * BUCKET
        tgts[bases[g]:bases[g] + caps[g]] = (g % N_BUCKET) * BUCKET
    srcs[slot_of_edge] = src
    tgts[slot_of_edge] = tgt

    wraps = []
    for (bs, bt) in _snake_groups():
        g = bs * N_BUCKET + bt
        off = 0
        for L in _segments(caps[g], seg_max):
            lo = bases[g] + off
            ws = _wrap16(srcs[lo:lo + L] - bs * BUCKET)   # [16, L/16]
            wt = _wrap16(tgts[lo:lo + L] - bt * BUCKET)
            wraps.append(np.concatenate([ws, ws, ws, ws, wt, wt, wt, wt]))
            off += L
    idx = np.concatenate(wraps, axis=1)

    eaT = np.zeros((EDGE_DIM, e_grid), ml_dtypes.bfloat16)
    eaT[:, slot_of_edge] = ea.T.astype(ml_dtypes.bfloat16)
    return idx, eaT, slot_of_edge


def _decode_out(o, e_grid):
    """[128, e_grid//2] packed -> [e_grid, 64] in slot order."""
    O = o.reshape(2, OUT_DIM, e_grid // 1024, 512)  # (j, f, t, q)
    return O.transpose(2, 0, 3, 1).reshape(e_grid, OUT_DIM)


_NC_CACHE = {}


def kernel(x, edge_attr, W1, b1, W2, b2, edge_index):
    global LAST_RESULT
    x = np.asarray(x, np.float32)
    edge_attr = np.asarray(edge_attr, np.float32)
    W1 = np.asarray(W1, np.float32)
    b1 = np.asarray(b1, np.float32)
    W2 = np.asarray(W2, np.float32)
    b2 = np.asarray(b2, np.float32)
    edge_index = np.asarray(edge_index)

    if "full" not in _NC_CACHE:
        _NC_CACHE["full"] = build_nc(N_NODES, GROUP_CAPS, SEG_MAX)
    nc = _NC_CACHE["full"]

    xtf = np.ascontiguousarray(x.T)
    w1ab = W1[:2 * NODE_DIM].astype(ml_dtypes.bfloat16)
    w1c = W1[2 * NODE_DIM:].astype(ml_dtypes.bfloat16)
    w2 = W2.astype(ml_dtypes.bfloat16)
    b1c = np.ascontiguousarray(b1.reshape(HIDDEN, 1))
    b2c = np.ascontiguousarray(b2.reshape(OUT_DIM, 1))

    src_all = edge_index[0].astype(np.int32)
    tgt_all = edge_index[1].astype(np.int32)

    in_maps = []
    slots = []
    for i in range(N_CORES):
        s, e = i * E_CORE, (i + 1) * E_CORE
        idx, eaT, slot = _prep_core(
            src_all[s:e], tgt_all[s:e], edge_attr[s:e],
            N_NODES, GROUP_CAPS, SEG_MAX,
        )
        slots.append(slot)
        in_maps.append({
            "xtf": xtf, "idx": idx, "eat": eaT,
            "w1ab": w1ab, "w1c": w1c, "w2": w2, "b1": b1c, "b2": b2c,
        })

    res = run_bass_kernel_spmd(
        nc, in_maps, core_ids=list(range(N_CORES)), trace=TRACE,
        tmpdir=TRACE_TMPDIR,
    )
    LAST_RESULT = res
    outs = []
    for i in range(N_CORES):
        y_slots = _decode_out(np.asarray(res.results[i]["out"]), E_GRID)
        outs.append(y_slots[slots[i]])
    return np.ascontiguousarray(np.concatenate(outs, axis=0), dtype=np.float32)
